# revision 20
# baseline (speedup 1.0000x reference)
"""Trainium2 Bass kernel for the CSTR (evaporator) 1M-step scan.

Parallel-in-time Picard: 1024 lanes (8 cores x 128 partitions) of L=1024
graded steps, each window extended K=192 spin-up steps back (slow mode
|lambda|~0.967 -> lambda^K ~ 1.5e-3 washes the wrong segment-entry state).
Two Picard-Gauss-Seidel sweeps over the nonlinear recurrence

  x0' = x0*(SC(u0) - c02*x0 - c03*x1) + c01
  x1' = SA(u1)*x1 + a10*x0 + SB(u0,u1)

with the linear-recurrence cores on the vector engine's tensor_tensor_scan.
den = 2*Cp*u1 + UA2 varies only +-0.5%, so 1/den is affine in u1 to 2.7e-5
rel: SA and the u1-part of SB collapse to single ACT affine ops (no
reciprocal). The host pre-gathers each core's 128 lane windows into a
chunk-major de-interleaved DRAM layout so input DMA is 4 fully-contiguous
pieces across both HWDGE queues and all SBUF reads are unit-stride.
Work split: DVE scans + c/v' STT chains, ACT affine singles, Pool
(b memset, cB2*u0 broadcast-mult, SB/a2 adds). Outputs stream out in 3
pieces per state. First L outputs are computed on host (segment 0 has no
spin-up protection). The compiled program bakes c01 (Pool memset), so the
build cache is keyed on c01.
"""

import numpy as np

T = 1048576
P = 128
NCORES = 8
L = 1024          # graded steps per lane
K = 192           # spin-up steps
W = K + L         # window length per lane (1216)
Wm = W - 1        # scan coefficient count (1215)
J0 = 64           # sweep-2 start column
TC = T // NCORES  # steps per core
SLAB = TC + K     # u rows staged per core
NCH = 4           # input DMA chunks
CW = W // NCH     # 304 cols per chunk
NC_CONST = 14

IN_CH = [(q * CW, (q + 1) * CW) for q in range(NCH)]
S2 = [(J0, 608), (608, 1060), (1060, Wm)]          # sweep-2 scan pieces
OUT_CH = [(max(lo + 1, K), hi + 1) for lo, hi in S2]  # output col ranges

# fixed model constants (match reference.py)
A, B, C_, D, E, F_, G, H = 0.5616, 0.3126, 48.43, 0.507, 55.0, 0.1538, 90.0, 0.16

_cache = {}


def _build_nc(c01_value):
    key = ("nc", float(c01_value))
    if key in _cache:
        return _cache[key]
    from contextlib import ExitStack
    import concourse.bacc as bacc
    import concourse.tile as tile
    import concourse.mybir as mybir
    from bass_rust import AP

    f32 = mybir.dt.float32
    bf16 = mybir.dt.bfloat16
    op = mybir.AluOpType
    ident = mybir.ActivationFunctionType.Identity
    nc = bacc.Bacc("TRN2", target_bir_lowering=False, debug=False,
                   enable_asserts=True, num_devices=NCORES)

    uslab = nc.dram_tensor("uslab", [NCH * 2 * P * CW], bf16, kind="ExternalInput").ap()
    cons = nc.dram_tensor("cons", [P, NC_CONST], f32, kind="ExternalInput").ap()
    o0 = nc.dram_tensor("o0", [P * L], f32, kind="ExternalOutput").ap()
    o1 = nc.dram_tensor("o1", [P * L], f32, kind="ExternalOutput").ap()

    with tile.TileContext(nc) as tc, ExitStack() as ctx:
        pool = ctx.enter_context(tc.tile_pool(name="main", bufs=1))
        t_u = pool.tile([P, 2 * W], bf16, tag="u")      # [u0 | u1] halves
        t_cons = pool.tile([P, NC_CONST], f32, tag="cons")
        t_SA = pool.tile([P, W], f32, tag="SA")
        t_SB = pool.tile([P, W], f32, tag="SB")
        t_tB = pool.tile([P, W], f32, tag="tB")         # cB2*u0
        t_a1 = pool.tile([P, W], f32, tag="a1")
        t_b = pool.tile([P, Wm], f32, tag="b")
        t_tmpa = pool.tile([P, Wm - J0], f32, tag="tmpa")
        t_SB2 = pool.tile([P, W], f32, tag="SB2")
        t_a2 = pool.tile([P, Wm - J0], f32, tag="a2")
        t_vm = pool.tile([P, Wm - J0], f32, tag="vm")
        t_c = pool.tile([P, Wm], f32, tag="c")
        t_v = pool.tile([P, Wm - J0], f32, tag="v")     # v' (sweep-2 a in t_a2)
        t_X0 = pool.tile([P, W], f32, tag="X0")
        t_X1 = pool.tile([P, W], f32, tag="X1")

        def cst(i):
            return t_cons[:, i : i + 1]

        u0 = t_u[:, 0:W]
        u1 = t_u[:, W : 2 * W]
        uap = t_u[:]
        u_pstride = uap.ap[0][0]
        cons_pstride = t_cons[:].ap[0][0]

        def cst_bcast(i, n):
            return AP(t_cons.tensor, t_cons[:].offset + i,
                      [[cons_pstride, P], [0, n]])

        # queue scalar: cons first (warms the ring), then input chunks 1,3
        nc.scalar.dma_start(t_cons[:], cons[:])
        # queue sync: tiny warm, then input chunks 0,2
        nc.sync.dma_start(t_tB[0:1, 0:4], cons[0:1, 0:4])
        for q, (lo, hi) in enumerate(IN_CH):
            eng = nc.sync if q % 2 == 0 else nc.scalar
            src = AP(uslab.tensor, q * 2 * P * CW,
                     [[CW, P], [P * CW, 2], [1, CW]])
            dst = AP(t_u.tensor, uap.offset + lo,
                     [[u_pstride, P], [W, 2], [1, CW]])
            eng.dma_start(dst, src)

        # ACT table warm-up while input DMA streams
        nc.scalar.activation(t_tB[:, 8:9], t_cons[:, 0:1], ident, bias=0.0, scale=1.0)
        # b tile: constant c01 (value baked; cache keyed on it)
        nc.gpsimd.memset(t_b[:], float(c01_value))
        nc.vector.tensor_copy(t_X0[:, 0:1], cst(12))
        nc.vector.tensor_copy(t_X1[:, 0:1], cst(13))

        # warm the DVE scan/STT pipeline and Pool TT library (first-use
        # cost ~700/300 ns) while the input DMA streams
        nc.vector.tensor_tensor_scan(t_tB[:, 16:20], t_cons[:, 0:4],
                                     t_cons[:, 0:4], t_cons[:, 0:1],
                                     op.mult, op.add)
        nc.gpsimd.tensor_tensor(t_tB[:, 24:28], t_cons[:, 0:4],
                                t_cons[:, 0:4], op.mult)

        # ---- precompute ----
        # ACT: SBu1 first then tB (both gate the SB chain), SA after;
        # Pool only does the SB adds (Pool reads bf16 at ~3.5 ns/col)
        for h0, h1 in [(0, 608), (608, W)]:
            nc.scalar.activation(t_SB[:, h0:h1], u1[:, h0:h1], ident,
                                 bias=cst(3), scale=cst(2))
            nc.scalar.activation(t_tB[:, h0:h1], u0[:, h0:h1], ident,
                                 bias=0.0, scale=cst(4))
            nc.scalar.activation(t_SA[:, h0:h1], u1[:, h0:h1], ident,
                                 bias=cst(1), scale=cst(0))
            if h0 == 0:
                nc.scalar.activation(t_a1[:, 608:1215], u0[:, 608:1215], ident,
                                     bias=cst(6), scale=cst(5))
            nc.gpsimd.tensor_tensor(t_SB2[:, h0:h1], t_tB[:, h0:h1],
                                    t_SB[:, h0:h1], op.add)

        # ---- sweep 1 (a1 on DVE tensor-scalar; c on DVE) ----
        nc.vector.tensor_scalar(t_a1[:, 0:608], u0[:, 0:608], cst(5), cst(6),
                                op.mult, op.add)
        nc.vector.tensor_tensor_scan(t_X0[:, 1:609], t_a1[:, 0:608],
                                     t_b[:, 0:608], t_X0[:, 0:1], op.mult, op.add)
        nc.vector.scalar_tensor_tensor(t_c[:, 0:608], t_X0[:, 0:608], cst(10),
                                       t_SB2[:, 0:608], op.mult, op.add)
        nc.vector.tensor_tensor_scan(t_X1[:, 1:609], t_SA[:, 0:608],
                                     t_c[:, 0:608], t_X1[:, 0:1], op.mult, op.add)
        nc.vector.scalar_tensor_tensor(t_v[:, 0:544], t_X0[:, 64:608], cst(9),
                                       t_a1[:, 64:608], op.mult, op.add)
        nc.vector.tensor_tensor_scan(t_X0[:, 609:1216], t_a1[:, 608:1215],
                                     t_b[:, 608:1215], t_X0[:, 608:609], op.mult, op.add)
        nc.vector.scalar_tensor_tensor(t_c[:, 608:1215], t_X0[:, 608:1215], cst(10),
                                       t_SB2[:, 608:1215], op.mult, op.add)
        nc.vector.tensor_tensor_scan(t_X1[:, 609:1216], t_SA[:, 608:1215],
                                     t_c[:, 608:1215], t_X1[:, 608:609], op.mult, op.add)
        nc.vector.scalar_tensor_tensor(t_v[:, 544:1151], t_X0[:, 608:1215], cst(9),
                                       t_a1[:, 608:1215], op.mult, op.add)

        # tmpa pieces (ACT); a2 = tmpa + v' runs on DVE just before use
        for lo, hi in [(J0, 608), (608, Wm)]:
            nc.scalar.activation(t_tmpa[:, lo - J0 : hi - J0], t_X1[:, lo:hi],
                                 ident, bias=cst(8), scale=cst(7))

        # ---- sweep 2 (DVE chain with per-piece output DMA) ----
        obase = [0]
        for lo, hi in S2:
            obase.append(obase[-1] + (hi + 1 - max(lo + 1, K)))
        nc.vector.tensor_tensor(t_a2[:, 0:544], t_tmpa[:, 0:544],
                                t_v[:, 0:544], op.add)
        nc.vector.tensor_tensor(t_a2[:, 544:1151], t_tmpa[:, 544:1151],
                                t_v[:, 544:1151], op.add)
        for p, (lo, hi) in enumerate(S2):
            nc.vector.tensor_tensor_scan(t_X0[:, lo + 1 : hi + 1],
                                         t_a2[:, lo - J0 : hi - J0],
                                         t_b[:, lo:hi], t_X0[:, lo : lo + 1],
                                         op.mult, op.add)
            if p == 0:
                nc.vector.scalar_tensor_tensor(t_c[:, lo:hi], t_X0[:, lo:hi],
                                               cst(10), t_SB2[:, lo:hi],
                                               op.mult, op.add)
            else:
                nc.scalar.activation(t_vm[:, lo - J0 : hi - J0], t_X0[:, lo:hi],
                                     ident, bias=0.0, scale=cst(10))
                nc.gpsimd.tensor_tensor(t_c[:, lo:hi],
                                        t_vm[:, lo - J0 : hi - J0],
                                        t_SB2[:, lo:hi], op.add)
            dlo, dhi = max(lo + 1, K), hi + 1
            cw = dhi - dlo
            dst0 = AP(o0.tensor, obase[p] * P, [[cw, P], [1, cw]])
            eng_o = nc.sync if p % 2 == 0 else nc.scalar
            eng_o.dma_start(dst0, t_X0[:, dlo:dhi])
            nc.vector.tensor_tensor_scan(t_X1[:, lo + 1 : hi + 1], t_SA[:, lo:hi],
                                         t_c[:, lo:hi], t_X1[:, lo : lo + 1],
                                         op.mult, op.add)
            dst1 = AP(o1.tensor, obase[p] * P, [[cw, P], [1, cw]])
            eng_o2 = nc.scalar if p % 2 == 0 else nc.sync
            eng_o2.dma_start(dst1, t_X1[:, dlo:dhi])

    nc.compile()
    _cache[key] = nc
    return nc


def _derive(params, x0, u1min, u1max):
    M, Cc, UA2, Cp, lam, lams, F1, X1p, F3, T1, T200 = [float(params[i]) for i in range(11)]
    UA1 = H * (F1 + F3)
    k1 = (UA1 + F1 * Cp) / lam
    p_ = k1 * B
    q_ = k1 * A
    alpha_u = UA1 * F_ / lam
    alpha_c = (UA1 * G + F1 * Cp * T1) / lam - k1 * C_
    c01 = F1 * X1p / M
    c02 = p_ / M
    c03 = q_ / M
    a10 = -p_ / Cc
    cA1 = 1.0 - q_ / Cc
    cA2 = -D / (lam * Cc)
    cB2 = alpha_u / Cc
    cB1 = alpha_c / Cc
    cB3 = -(E - T200) / (lam * Cc)
    cC2 = alpha_u / M
    cC1 = 1.0 - (F1 - alpha_c) / M
    i0, i1 = float(x0[0]), float(x0[1])
    # affine reciprocal of den = 2*Cp*u1 + UA2 over the actual u1 range
    denc = 0.5 * ((2 * Cp * u1min + UA2) + (2 * Cp * u1max + UA2))
    ra = -2.0 * Cp / (denc * denc)
    rb = (2.0 * denc - UA2) / (denc * denc)
    sa1 = -cA2 * UA2 * UA2
    sa0 = cA1 + cA2 * UA2
    sb1 = -cB3 * UA2 * UA2
    sb0 = cB1 + cB3 * UA2
    const10 = c02 * i0 + c03 * i1

    cv = np.zeros(NC_CONST, np.float64)
    cv[0] = sa1 * ra                 # SA scale (u1)
    cv[1] = sa0 + sa1 * rb           # SA bias
    cv[2] = sb1 * ra                 # SBu1 scale (u1)
    cv[3] = sb0 + sb1 * rb           # SBu1 bias
    cv[4] = cB2                      # tB broadcast scalar (u0)
    cv[5] = cC2                      # a1 scale (u0)
    cv[6] = cC1 - const10            # a1 bias
    cv[7] = -c03                     # tmpa scale (X1)
    cv[8] = const10                  # tmpa bias
    cv[9] = -c02                     # v' scalar
    cv[10] = a10                     # c scalar
    cv[11] = c01                     # (baked in memset; for reference)
    cv[12] = i0
    cv[13] = i1
    return cv.astype(np.float32), np.float32(c01)


def _make_in_maps(u, cons):
    import ml_dtypes
    bf16 = ml_dtypes.bfloat16
    u = np.ascontiguousarray(u, np.float32)
    consT = np.tile(cons[None, :], (P, 1))
    lane_rows = np.arange(P)[:, None] * L  # [P,1]
    in_maps = []
    for c in range(NCORES):
        if c == 0:
            slab = np.concatenate([np.repeat(u[0:1], K, axis=0), u[0:TC]], axis=0)
        else:
            slab = u[c * TC - K : c * TC + TC]
        parts = []
        for lo, hi in IN_CH:
            rows = lane_rows + np.arange(lo, hi)[None, :]  # [P,CW]
            blk = slab[rows].astype(bf16)                  # [P,CW,2] bf16
            parts.append(np.ascontiguousarray(blk[:, :, 0]).ravel())
            parts.append(np.ascontiguousarray(blk[:, :, 1]).ravel())
        in_maps.append({
            "uslab": np.concatenate(parts),
            "cons": consT,
        })
    return in_maps


def _host_head(u, x0, params, n):
    # exact fp32 simulation of the first n steps (segment 0 has no spin-up)
    f = np.float32
    M, Cc, UA2, Cp, lam, lams, F1, X1p, F3, T1, T200 = [f(params[i]) for i in range(11)]
    out = np.empty((n, 2), f)
    s0, s1 = f(x0[0]), f(x0[1])
    fA, fB, fC, fD, fE, fF, fG, fH = f(A), f(B), f(C_), f(D), f(E), f(F_), f(G), f(H)
    one, two = f(1.0), f(2.0)
    UA1 = fH * (F1 + F3)
    for t in range(n):
        out[t, 0] = s0
        out[t, 1] = s1
        u0, u1 = f(u[t, 0]), f(u[t, 1])
        T2 = fA * s1 + fB * s0 + fC
        T3 = fD * s1 + fE
        T100 = fF * u0 + fG
        Q100 = UA1 * (T100 - T2)
        Q200 = UA2 * (T3 - T200) / (one + UA2 / (two * Cp * u1))
        F5 = Q200 / lam
        F4 = (Q100 - F1 * Cp * (T2 - T1)) / lam
        F2 = F1 - F4
        X2d = (F1 * X1p - F2 * s0) / M
        P2d = (F4 - F5) / Cc
        s0 = s0 + X2d
        s1 = s1 + P2d
    return out


def _assemble(results, head):
    widths = [hi - lo for lo, hi in OUT_CH]
    bounds = np.cumsum([0] + [w * P for w in widths])
    out = np.empty((T, 2), np.float32)
    for c in range(NCORES):
        for k, name in enumerate(("o0", "o1")):
            flat = results[c][name].reshape(-1)
            cols = [flat[bounds[q] : bounds[q + 1]].reshape(P, widths[q])
                    for q in range(len(widths))]
            out[c * TC : (c + 1) * TC, k] = np.concatenate(cols, axis=1).reshape(-1)
    out[0:L] = head
    return out


def run(u_forced, x0, params, trace=False):
    from concourse.bass_utils import run_bass_kernel_spmd
    u = np.ascontiguousarray(u_forced, np.float32)
    u1min = float(u[:, 1].min())
    u1max = float(u[:, 1].max())
    cons, c01 = _derive(params, x0, u1min, u1max)
    nc = _build_nc(c01)
    in_maps = _make_in_maps(u, cons)
    head = _host_head(u, x0, params, L)
    res = run_bass_kernel_spmd(nc, in_maps, list(range(NCORES)), trace=trace)
    return _assemble(res.results, head), res


def kernel(u_forced, x0, params):
    out, _ = run(u_forced, x0, params, trace=False)
    return out


# revision 21
# speedup vs baseline: 1.0275x; 1.0275x over previous
"""Trainium2 Bass kernel for the CSTR (evaporator) 1M-step scan.

Parallel-in-time Picard: 1024 lanes (8 cores x 128 partitions) of L=1024
graded steps, each window extended K=192 spin-up steps back (slow mode
|lambda|~0.967 -> lambda^K ~ 1.5e-3 washes the wrong segment-entry state).
Two Picard-Gauss-Seidel sweeps over the nonlinear recurrence

  x0' = x0*(SC(u0) - c02*x0 - c03*x1) + c01
  x1' = SA(u1)*x1 + a10*x0 + SB(u0,u1)

with the linear-recurrence cores on the vector engine's tensor_tensor_scan.
den = 2*Cp*u1 + UA2 varies only +-0.5%, so 1/den is affine in u1 to 2.7e-5
rel: SA and the u1-part of SB collapse to single ACT affine ops (no
reciprocal). The host pre-gathers each core's 128 lane windows into a
chunk-major de-interleaved DRAM layout so input DMA is 4 fully-contiguous
pieces across both HWDGE queues and all SBUF reads are unit-stride.
Work split: DVE scans + c/v' STT chains, ACT affine singles, Pool
(b memset, cB2*u0 broadcast-mult, SB/a2 adds). Outputs stream out in 3
pieces per state. First L outputs are computed on host (segment 0 has no
spin-up protection). The compiled program bakes c01 (Pool memset), so the
build cache is keyed on c01.
"""

import numpy as np

T = 1048576
P = 128
NCORES = 8
L = 1024          # graded steps per lane
K = 192           # spin-up steps
W = K + L         # window length per lane (1216)
Wm = W - 1        # scan coefficient count (1215)
J0 = 64           # sweep-2 start column
TC = T // NCORES  # steps per core
SLAB = TC + K     # u rows staged per core
NCH = 4           # input DMA chunks
CW = W // NCH     # 304 cols per chunk
NC_CONST = 14

IN_CH = [(q * CW, (q + 1) * CW) for q in range(NCH)]
S2 = [(J0, 608), (608, 1060), (1060, Wm)]          # sweep-2 scan pieces
OUT_CH = [(max(lo + 1, K), hi + 1) for lo, hi in S2]  # output col ranges

# fixed model constants (match reference.py)
A, B, C_, D, E, F_, G, H = 0.5616, 0.3126, 48.43, 0.507, 55.0, 0.1538, 90.0, 0.16

_cache = {}


def _build_nc(c01_value):
    key = ("nc", float(c01_value))
    if key in _cache:
        return _cache[key]
    from contextlib import ExitStack
    import concourse.bacc as bacc
    import concourse.tile as tile
    import concourse.mybir as mybir
    from bass_rust import AP

    f32 = mybir.dt.float32
    bf16 = mybir.dt.bfloat16
    op = mybir.AluOpType
    ident = mybir.ActivationFunctionType.Identity
    nc = bacc.Bacc("TRN2", target_bir_lowering=False, debug=False,
                   enable_asserts=True, num_devices=NCORES)

    uslab = nc.dram_tensor("uslab", [NCH * 2 * P * CW], bf16, kind="ExternalInput").ap()
    cons = nc.dram_tensor("cons", [P, NC_CONST], f32, kind="ExternalInput").ap()
    o0 = nc.dram_tensor("o0", [P * L], f32, kind="ExternalOutput").ap()
    o1 = nc.dram_tensor("o1", [P * L], f32, kind="ExternalOutput").ap()

    with tile.TileContext(nc) as tc, ExitStack() as ctx:
        pool = ctx.enter_context(tc.tile_pool(name="main", bufs=1))
        t_u = pool.tile([P, 2 * W], bf16, tag="u")      # [u0 | u1] halves
        t_cons = pool.tile([P, NC_CONST], f32, tag="cons")
        t_SA = pool.tile([P, W], f32, tag="SA")
        t_SB = pool.tile([P, W], f32, tag="SB")
        t_tB = pool.tile([P, W], f32, tag="tB")         # cB2*u0
        t_a1 = pool.tile([P, W], f32, tag="a1")
        t_b = pool.tile([P, Wm], f32, tag="b")
        t_tmpa = pool.tile([P, Wm - J0], f32, tag="tmpa")
        t_SB2 = pool.tile([P, W], f32, tag="SB2")
        t_a2 = pool.tile([P, Wm - J0], f32, tag="a2")
        t_vm = pool.tile([P, Wm - J0], f32, tag="vm")
        t_c = pool.tile([P, Wm], f32, tag="c")
        t_v = pool.tile([P, Wm - J0], f32, tag="v")     # v' (sweep-2 a in t_a2)
        t_X0 = pool.tile([P, W], f32, tag="X0")
        t_X1 = pool.tile([P, W], f32, tag="X1")

        def cst(i):
            return t_cons[:, i : i + 1]

        u0 = t_u[:, 0:W]
        u1 = t_u[:, W : 2 * W]
        uap = t_u[:]
        u_pstride = uap.ap[0][0]
        cons_pstride = t_cons[:].ap[0][0]

        def cst_bcast(i, n):
            return AP(t_cons.tensor, t_cons[:].offset + i,
                      [[cons_pstride, P], [0, n]])

        # queue scalar: cons first (warms the ring), then input chunks 1,3
        nc.scalar.dma_start(t_cons[:], cons[:])
        # queue sync: tiny warm, then input chunks 0,2
        nc.sync.dma_start(t_tB[0:1, 0:4], cons[0:1, 0:4])
        for q, (lo, hi) in enumerate(IN_CH):
            eng = nc.sync if q % 2 == 0 else nc.scalar
            src = AP(uslab.tensor, q * 2 * P * CW,
                     [[CW, P], [P * CW, 2], [1, CW]])
            dst = AP(t_u.tensor, uap.offset + lo,
                     [[u_pstride, P], [W, 2], [1, CW]])
            eng.dma_start(dst, src)

        # ACT table warm-up while input DMA streams
        nc.scalar.activation(t_tB[:, 8:9], t_cons[:, 0:1], ident, bias=0.0, scale=1.0)
        # b tile: constant c01 (value baked; cache keyed on it)
        nc.gpsimd.memset(t_b[:], float(c01_value))
        nc.vector.tensor_copy(t_X0[:, 0:1], cst(12))
        nc.vector.tensor_copy(t_X1[:, 0:1], cst(13))

        # warm the DVE scan/STT pipeline and Pool TT library (first-use
        # cost ~700/300 ns) while the input DMA streams
        nc.vector.tensor_tensor_scan(t_tB[:, 16:20], t_cons[:, 0:4],
                                     t_cons[:, 0:4], t_cons[:, 0:1],
                                     op.mult, op.add)
        nc.gpsimd.tensor_tensor(t_tB[:, 24:28], t_cons[:, 0:4],
                                t_cons[:, 0:4], op.mult)

        # ---- precompute ----
        # ACT: SBu1 first then tB (both gate the SB chain), SA after;
        # Pool only does the SB adds (Pool reads bf16 at ~3.5 ns/col)
        for h0, h1 in [(0, 608), (608, W)]:
            nc.scalar.activation(t_SB[:, h0:h1], u1[:, h0:h1], ident,
                                 bias=cst(3), scale=cst(2))
            nc.scalar.activation(t_tB[:, h0:h1], u0[:, h0:h1], ident,
                                 bias=0.0, scale=cst(4))
            nc.scalar.activation(t_SA[:, h0:h1], u1[:, h0:h1], ident,
                                 bias=cst(1), scale=cst(0))
            if h0 == 0:
                nc.scalar.activation(t_a1[:, 608:1215], u0[:, 608:1215], ident,
                                     bias=cst(6), scale=cst(5))
            nc.gpsimd.tensor_tensor(t_SB2[:, h0:h1], t_tB[:, h0:h1],
                                    t_SB[:, h0:h1], op.add)

        # ---- sweep 1 (a1 on DVE tensor-scalar; c on DVE) ----
        nc.vector.tensor_scalar(t_a1[:, 0:608], u0[:, 0:608], cst(5), cst(6),
                                op.mult, op.add)
        nc.vector.tensor_tensor_scan(t_X0[:, 1:609], t_a1[:, 0:608],
                                     t_b[:, 0:608], t_X0[:, 0:1], op.mult, op.add)
        nc.vector.scalar_tensor_tensor(t_c[:, 0:608], t_X0[:, 0:608], cst(10),
                                       t_SB2[:, 0:608], op.mult, op.add)
        nc.vector.tensor_tensor_scan(t_X1[:, 1:609], t_SA[:, 0:608],
                                     t_c[:, 0:608], t_X1[:, 0:1], op.mult, op.add)
        nc.vector.scalar_tensor_tensor(t_v[:, 0:544], t_X0[:, 64:608], cst(9),
                                       t_a1[:, 64:608], op.mult, op.add)
        nc.vector.tensor_tensor_scan(t_X0[:, 609:1216], t_a1[:, 608:1215],
                                     t_b[:, 608:1215], t_X0[:, 608:609], op.mult, op.add)
        nc.vector.scalar_tensor_tensor(t_c[:, 608:1215], t_X0[:, 608:1215], cst(10),
                                       t_SB2[:, 608:1215], op.mult, op.add)
        nc.vector.tensor_tensor_scan(t_X1[:, 609:1216], t_SA[:, 608:1215],
                                     t_c[:, 608:1215], t_X1[:, 608:609], op.mult, op.add)
        nc.vector.scalar_tensor_tensor(t_v[:, 544:1151], t_X0[:, 608:1215], cst(9),
                                       t_a1[:, 608:1215], op.mult, op.add)

        # tmpa pieces (ACT); a2 = tmpa + v' runs on DVE just before use
        for lo, hi in [(J0, 608), (608, Wm)]:
            nc.scalar.activation(t_tmpa[:, lo - J0 : hi - J0], t_X1[:, lo:hi],
                                 ident, bias=cst(8), scale=cst(7))

        # ---- sweep 2 (DVE chain with per-piece output DMA) ----
        obase = [0]
        for lo, hi in S2:
            obase.append(obase[-1] + (hi + 1 - max(lo + 1, K)))
        nc.vector.tensor_tensor(t_a2[:, 0:544], t_tmpa[:, 0:544],
                                t_v[:, 0:544], op.add)
        nc.vector.tensor_tensor(t_a2[:, 544:1151], t_tmpa[:, 544:1151],
                                t_v[:, 544:1151], op.add)
        for p, (lo, hi) in enumerate(S2):
            nc.vector.tensor_tensor_scan(t_X0[:, lo + 1 : hi + 1],
                                         t_a2[:, lo - J0 : hi - J0],
                                         t_b[:, lo:hi], t_X0[:, lo : lo + 1],
                                         op.mult, op.add)
            nc.vector.scalar_tensor_tensor(t_c[:, lo:hi], t_X0[:, lo:hi],
                                           cst(10), t_SB2[:, lo:hi],
                                           op.mult, op.add)
            dlo, dhi = max(lo + 1, K), hi + 1
            cw = dhi - dlo
            dst0 = AP(o0.tensor, obase[p] * P, [[cw, P], [1, cw]])
            eng_o = nc.sync if p % 2 == 0 else nc.scalar
            eng_o.dma_start(dst0, t_X0[:, dlo:dhi])
            nc.vector.tensor_tensor_scan(t_X1[:, lo + 1 : hi + 1], t_SA[:, lo:hi],
                                         t_c[:, lo:hi], t_X1[:, lo : lo + 1],
                                         op.mult, op.add)
            dst1 = AP(o1.tensor, obase[p] * P, [[cw, P], [1, cw]])
            eng_o2 = nc.scalar if p % 2 == 0 else nc.sync
            eng_o2.dma_start(dst1, t_X1[:, dlo:dhi])

    nc.compile()
    _cache[key] = nc
    return nc


def _derive(params, x0, u1min, u1max):
    M, Cc, UA2, Cp, lam, lams, F1, X1p, F3, T1, T200 = [float(params[i]) for i in range(11)]
    UA1 = H * (F1 + F3)
    k1 = (UA1 + F1 * Cp) / lam
    p_ = k1 * B
    q_ = k1 * A
    alpha_u = UA1 * F_ / lam
    alpha_c = (UA1 * G + F1 * Cp * T1) / lam - k1 * C_
    c01 = F1 * X1p / M
    c02 = p_ / M
    c03 = q_ / M
    a10 = -p_ / Cc
    cA1 = 1.0 - q_ / Cc
    cA2 = -D / (lam * Cc)
    cB2 = alpha_u / Cc
    cB1 = alpha_c / Cc
    cB3 = -(E - T200) / (lam * Cc)
    cC2 = alpha_u / M
    cC1 = 1.0 - (F1 - alpha_c) / M
    i0, i1 = float(x0[0]), float(x0[1])
    # affine reciprocal of den = 2*Cp*u1 + UA2 over the actual u1 range
    denc = 0.5 * ((2 * Cp * u1min + UA2) + (2 * Cp * u1max + UA2))
    ra = -2.0 * Cp / (denc * denc)
    rb = (2.0 * denc - UA2) / (denc * denc)
    sa1 = -cA2 * UA2 * UA2
    sa0 = cA1 + cA2 * UA2
    sb1 = -cB3 * UA2 * UA2
    sb0 = cB1 + cB3 * UA2
    const10 = c02 * i0 + c03 * i1

    cv = np.zeros(NC_CONST, np.float64)
    cv[0] = sa1 * ra                 # SA scale (u1)
    cv[1] = sa0 + sa1 * rb           # SA bias
    cv[2] = sb1 * ra                 # SBu1 scale (u1)
    cv[3] = sb0 + sb1 * rb           # SBu1 bias
    cv[4] = cB2                      # tB broadcast scalar (u0)
    cv[5] = cC2                      # a1 scale (u0)
    cv[6] = cC1 - const10            # a1 bias
    cv[7] = -c03                     # tmpa scale (X1)
    cv[8] = const10                  # tmpa bias
    cv[9] = -c02                     # v' scalar
    cv[10] = a10                     # c scalar
    cv[11] = c01                     # (baked in memset; for reference)
    cv[12] = i0
    cv[13] = i1
    return cv.astype(np.float32), np.float32(c01)


def _make_in_maps(u, cons):
    import ml_dtypes
    bf16 = ml_dtypes.bfloat16
    u = np.ascontiguousarray(u, np.float32)
    consT = np.tile(cons[None, :], (P, 1))
    lane_rows = np.arange(P)[:, None] * L  # [P,1]
    in_maps = []
    for c in range(NCORES):
        if c == 0:
            slab = np.concatenate([np.repeat(u[0:1], K, axis=0), u[0:TC]], axis=0)
        else:
            slab = u[c * TC - K : c * TC + TC]
        parts = []
        for lo, hi in IN_CH:
            rows = lane_rows + np.arange(lo, hi)[None, :]  # [P,CW]
            blk = slab[rows].astype(bf16)                  # [P,CW,2] bf16
            parts.append(np.ascontiguousarray(blk[:, :, 0]).ravel())
            parts.append(np.ascontiguousarray(blk[:, :, 1]).ravel())
        in_maps.append({
            "uslab": np.concatenate(parts),
            "cons": consT,
        })
    return in_maps


def _host_head(u, x0, params, n):
    # exact fp32 simulation of the first n steps (segment 0 has no spin-up)
    f = np.float32
    M, Cc, UA2, Cp, lam, lams, F1, X1p, F3, T1, T200 = [f(params[i]) for i in range(11)]
    out = np.empty((n, 2), f)
    s0, s1 = f(x0[0]), f(x0[1])
    fA, fB, fC, fD, fE, fF, fG, fH = f(A), f(B), f(C_), f(D), f(E), f(F_), f(G), f(H)
    one, two = f(1.0), f(2.0)
    UA1 = fH * (F1 + F3)
    for t in range(n):
        out[t, 0] = s0
        out[t, 1] = s1
        u0, u1 = f(u[t, 0]), f(u[t, 1])
        T2 = fA * s1 + fB * s0 + fC
        T3 = fD * s1 + fE
        T100 = fF * u0 + fG
        Q100 = UA1 * (T100 - T2)
        Q200 = UA2 * (T3 - T200) / (one + UA2 / (two * Cp * u1))
        F5 = Q200 / lam
        F4 = (Q100 - F1 * Cp * (T2 - T1)) / lam
        F2 = F1 - F4
        X2d = (F1 * X1p - F2 * s0) / M
        P2d = (F4 - F5) / Cc
        s0 = s0 + X2d
        s1 = s1 + P2d
    return out


def _assemble(results, head):
    widths = [hi - lo for lo, hi in OUT_CH]
    bounds = np.cumsum([0] + [w * P for w in widths])
    out = np.empty((T, 2), np.float32)
    for c in range(NCORES):
        for k, name in enumerate(("o0", "o1")):
            flat = results[c][name].reshape(-1)
            cols = [flat[bounds[q] : bounds[q + 1]].reshape(P, widths[q])
                    for q in range(len(widths))]
            out[c * TC : (c + 1) * TC, k] = np.concatenate(cols, axis=1).reshape(-1)
    out[0:L] = head
    return out


def run(u_forced, x0, params, trace=False):
    from concourse.bass_utils import run_bass_kernel_spmd
    u = np.ascontiguousarray(u_forced, np.float32)
    u1min = float(u[:, 1].min())
    u1max = float(u[:, 1].max())
    cons, c01 = _derive(params, x0, u1min, u1max)
    nc = _build_nc(c01)
    in_maps = _make_in_maps(u, cons)
    head = _host_head(u, x0, params, L)
    res = run_bass_kernel_spmd(nc, in_maps, list(range(NCORES)), trace=trace)
    return _assemble(res.results, head), res


def kernel(u_forced, x0, params):
    out, _ = run(u_forced, x0, params, trace=False)
    return out
